# revision 35
# baseline (speedup 1.0000x reference)
"""Trainium2 Bass kernel for nn_CollisonToJointLoss.

Math restructure (same identity as the earlier version): jr >= 0, so where
both gathered scores are nonzero, |intr_s + recv_s| = intr_s + recv_s, and

    num_b = <D_b, Sum_c Sum_t [S|M]_intr^T [S|M]_recv  (TR + BL blocks)>
    den_b = sum(BR block),    with S_v = jr[v], M_v = (S_v > 0).

Key layout change vs the 104us version: the gather table is indexed BY FACE,
not by vertex.  Each 512B table row holds the face's three [S|M] vertex rows
([3 x 48] bf16 = 288B used).  One collision side therefore costs ONE gather
descriptor instead of 1 (face->verts) + 3 (vert->jr) descriptors of 256B
each: 8192 descriptors total instead of 32768, i.e. ~11.7us of DMA instead
of ~46.6us under the 22.76ns/descriptor DMA cost (256B and 512B descriptors
cost the same; the floor is at work per descriptor, not bytes).

The gather index array is host-prepared in the HW wrapped layout
([16, n/16] int16, replicated across the eight 16-partition groups), with
invalid collisions (cf[:,0] < 0) redirected to an all-zero table row and the
per-batch table base (+b*FPAD) folded in.  The gather runs as 4 chunks of
2048 descriptors so PE accumulation overlaps later chunks, and batch 0's
final reduction overlaps batch 1's gather.

Sharding: data-parallel over batch B: 8 cores x 2 batches.  Each core
returns partial (num, den); host sums and finishes the mean.
"""

import numpy as np

B, C, N, F, J = 16, 2048, 6890, 13776, 24
NCORES = 8
BPC = B // NCORES          # batches per core
NPAD = 6912                # 128 * 54  (jr/verts padded with zero rows)
KCH = NPAD // 128          # 54 chunks for the joints matmul
FPAD = F + 1               # table rows per batch incl. zero-face row
J2 = 2 * J                 # 48
E = 256                    # table row width in bf16 elems (512B stride)
NIDX = BPC * C * 2         # 8192 gather descriptors per core
NCHUNK = 4
CIDX = NIDX // NCHUNK      # 2048 descriptors per gather chunk

_CACHE = {}


def _build_program():
    import concourse.bass as bass
    import concourse.tile as tile
    from concourse import bacc, mybir
    from concourse.masks import make_identity

    f32 = mybir.dt.float32
    bf16 = mybir.dt.bfloat16
    i16 = mybir.dt.int16
    Alu = mybir.AluOpType

    nc = bacc.Bacc("TRN2", target_bir_lowering=False, debug=False,
                   num_swdge_queues=2)

    widx_d = nc.dram_tensor("widx", [128, NIDX // 16], i16,
                            kind="ExternalInput").ap()
    jrt_d = nc.dram_tensor("jrt", [128, KCH * J], bf16,
                           kind="ExternalInput").ap()
    vc_d = nc.dram_tensor("vc", [128, KCH * 6], bf16,
                          kind="ExternalInput").ap()
    fsm_d = nc.dram_tensor("fsm", [BPC * FPAD, E], bf16,
                           kind="ExternalInput").ap()
    msk_d = nc.dram_tensor("msk", [J2, J2], f32, kind="ExternalInput").ap()
    out_d = nc.dram_tensor("out", [J2, 4], f32, kind="ExternalOutput").ap()

    with tile.TileContext(nc) as tc:
        with tc.tile_pool(name="sb", bufs=1) as sb, \
             tc.tile_pool(name="pp", bufs=1, space="PSUM") as pp:

            # ---- bulk loads (widx first: it gates the gather pipeline).
            # The first chunk's index columns load separately so its
            # descriptor generation starts ~270ns earlier.
            WIDX = sb.tile([128, NIDX // 16], i16)
            nc.sync.dma_start(out=WIDX[:, 0:80], in_=widx_d[:, 0:80])
            nc.sync.dma_start(out=WIDX[:, 80:], in_=widx_d[:, 80:])
            JT = sb.tile([128, KCH, J], bf16)
            nc.sync.dma_start(out=JT[:].rearrange("p k j -> p (k j)"),
                              in_=jrt_d)
            VC = sb.tile([128, KCH, 6], bf16)
            nc.sync.dma_start(out=VC[:].rearrange("p k d -> p (k d)"),
                              in_=vc_d)

            MSK = sb.tile([J2, J2], f32)
            nc.sync.dma_start(out=MSK[:], in_=msk_d)
            ident = sb.tile([128, 128], f32)
            make_identity(nc, ident[:])

            # ---- joints = jr^T-chunks contracted with verts ----------------
            J6p = pp.tile([J, 6], f32)
            for k in range(KCH):
                nc.tensor.matmul(out=J6p[:], lhsT=JT[:, k, :], rhs=VC[:, k, :],
                                 start=(k == 0), stop=(k == KCH - 1))
            j6 = sb.tile([J, 6], f32)
            nc.vector.tensor_copy(out=j6[:], in_=J6p[:])

            # jt_b^T [3, 24] duplicated to [3, 48] so the squared-distance
            # matrix lands on all 48 partitions directly (D48 blocks = D).
            jtp = pp.tile([3, J2], f32)
            for b in range(BPC):
                nc.tensor.transpose(out=jtp[:, J * b:J * b + J],
                                    in_=j6[:, 3 * b:3 * b + 3],
                                    identity=ident[:J, :J])
            jtd = [sb.tile([3, J2], f32, name=f"jtd{b}") for b in range(BPC)]
            sqd = [sb.tile([3, J2], f32, name=f"sqd{b}") for b in range(BPC)]
            jtm2 = [sb.tile([3, J2], f32, name=f"jtm2{b}") for b in range(BPC)]
            ones3_48 = sb.tile([3, J2], f32)
            nc.vector.memset(ones3_48[:], 1.0)
            for b in range(BPC):
                nc.vector.tensor_copy(out=jtd[b][:, 0:J],
                                      in_=jtp[:, J * b:J * b + J])
                nc.vector.tensor_copy(out=jtd[b][:, J:J2],
                                      in_=jtp[:, J * b:J * b + J])
                nc.vector.tensor_mul(out=sqd[b][:], in0=jtd[b][:],
                                     in1=jtd[b][:])
                nc.vector.tensor_scalar_mul(out=jtm2[b][:], in0=jtd[b][:],
                                            scalar1=-2.0)

            # ---- DD_b: pairwise joint distances on 48 partitions, with the
            # diagonal blocks zeroed (only TR/BL of ACC contribute to num)
            G48 = pp.tile([J2, J2], f32)
            DD = [sb.tile([J2, J2], f32, name=f"DD{b}") for b in range(BPC)]
            for b in range(BPC):
                nc.tensor.matmul(out=G48[:], lhsT=jtm2[b][:], rhs=jtd[b][:],
                                 start=True, stop=False)
                nc.tensor.matmul(out=G48[:], lhsT=ones3_48[:], rhs=sqd[b][:],
                                 start=False, stop=False)
                nc.tensor.matmul(out=G48[:], lhsT=sqd[b][:], rhs=ones3_48[:],
                                 start=False, stop=True)
                nc.vector.tensor_scalar_max(out=DD[b][:], in0=G48[:],
                                            scalar1=0.0)
                nc.scalar.activation(out=DD[b][:], in_=DD[b][:],
                                     func=mybir.ActivationFunctionType.Sqrt)
                nc.vector.tensor_mul(out=DD[b][:], in0=DD[b][:], in1=MSK[:])

            # ---- chunked face-row gathers ---------------------------------
            # descriptor k = T*256 + side*128 + p; table row holds the face's
            # 3 [S|M] vertex rows at 48-elem offsets.  U slot = 2T + side.
            # Chunk sizes taper so the post-gather tail is short; desc-gen
            # (994 + 0.34/desc) stays ahead of the 1.42ns/desc transfers.
            U = sb.tile([128, NIDX // 128, E], bf16)
            bounds = [0, 1280, 3328, 5376, 7936, 8192]
            for ch in range(len(bounds) - 1):
                k0, k1 = bounds[ch], bounds[ch + 1]
                nc.gpsimd.dma_gather(
                    out_ap=U[:, k0 // 128:k1 // 128, :], in_ap=fsm_d,
                    idxs_ap=WIDX[:, k0 // 16:k1 // 16],
                    num_idxs=k1 - k0, num_idxs_reg=k1 - k0, elem_size=E,
                    single_packet=False, queue_num=ch % 2)

            # ---- accumulate ACC_b = Sum [S|M]_intr^T [S|M]_recv -----------
            # tiles T 0-15 are batch 0, 16-31 batch 1 (chunks in T order).
            # Each batch's reduction is emitted right after its last tile so
            # batch 0's reduction overlaps the remaining gather chunks.
            ACC = [pp.tile([J2, J2], f32, name=f"ACC{b}") for b in range(BPC)]
            VV = sb.tile([J2, 4], f32)
            nc.vector.memset(VV[:], 0.0)

            def reduction(b):
                NU = sb.tile([J2, J2], f32, name=f"NU{b}")
                nc.vector.tensor_mul(out=NU[:], in0=ACC[b][:], in1=DD[b][:])
                nc.vector.reduce_sum(out=VV[:, 2 * b:2 * b + 1], in_=NU[:],
                                     axis=mybir.AxisListType.X)
                nc.vector.reduce_sum(out=VV[:, 2 * b + 1:2 * b + 2],
                                     in_=ACC[b][:, J:J2],
                                     axis=mybir.AxisListType.X)

            started = [False, False]
            for T in range(32):
                b = T // 16
                for tau in range(3):
                    nc.tensor.matmul(
                        out=ACC[b][:],
                        lhsT=U[:, 2 * T + 1, J2 * tau:J2 * (tau + 1)],
                        rhs=U[:, 2 * T, J2 * tau:J2 * (tau + 1)],
                        start=not started[b],
                        stop=(T % 16 == 15 and tau == 2))
                    started[b] = True
                if T % 16 == 15:
                    reduction(b)
            nc.sync.dma_start(out=out_d, in_=VV[:])

    nc.compile()
    return nc


def get_program():
    if "nc" not in _CACHE:
        _CACHE["nc"] = _build_program()
    return _CACHE["nc"]


def make_in_maps(collision_idxs, vertices, faces, joint_regressor):
    """Host-side shard/layout prep. Returns list of per-core input dicts."""
    import ml_dtypes
    bf16 = ml_dtypes.bfloat16

    collision_idxs = np.asarray(collision_idxs)
    vertices = np.asarray(vertices)
    faces = np.asarray(faces).astype(np.int64)
    joint_regressor = np.asarray(joint_regressor)

    # jr^T padded, f32 for the joints matmul; [S|M] rows in bf16 for the table
    jrt = np.zeros((NPAD, J), dtype=np.float32)
    jrt[:N, :] = joint_regressor.T.astype(np.float32)
    sm = np.zeros((NPAD, J2), dtype=bf16)
    sm[:N, 0:J] = jrt[:N].astype(bf16)
    sm[:N, J:J2] = (jrt[:N] != 0).astype(bf16)

    # per-(batch, face) table row: 3 x [S|M] = 144 bf16, padded to 256
    fsm_all = np.zeros((B, FPAD, E), dtype=bf16)
    fsm_all[:, :F, 0:3 * J2] = sm[faces.reshape(B, F * 3)].reshape(B, F, 3 * J2)

    vpad = np.zeros((B, NPAD, 3), dtype=np.float32)
    vpad[:, :N, :] = vertices.astype(np.float32)

    # gather index values: valid ? clip(cf) : F (zero row), + b*FPAD
    cidx = collision_idxs.astype(np.int32)
    valid = cidx[:, :, 0] >= 0
    sel = np.empty((2, B, C), dtype=np.int32)      # side 0 = recv, 1 = intr
    sel[0] = np.where(valid, np.maximum(cidx[:, :, 0], 0), F)
    sel[1] = np.where(valid, np.maximum(cidx[:, :, 1], 0), F)

    # c(q, t, a) = q*128 + t*8 + a; descriptor k = T*256 + side*128 + 16a + q
    cgrid = (np.arange(16)[:, None, None] * 128 +
             np.arange(16)[None, :, None] * 8 +
             np.arange(8)[None, None, :])          # [q, t, a]

    in_maps = []
    for core in range(NCORES):
        bs = slice(core * BPC, (core + 1) * BPC)
        v = np.empty((2 * 16, 2, 8, 16), dtype=np.int32)   # [T, side, a, q]
        for bb in range(BPC):
            for side in range(2):
                g = sel[side, core * BPC + bb][cgrid]      # [q, t, a]
                v[bb * 16:(bb + 1) * 16, side] = (
                    bb * FPAD + g.transpose(1, 2, 0))      # [t, a, q]
        wrapped = v.reshape(NIDX // 16, 16).T              # [q, slot]
        widx = np.tile(wrapped, (8, 1)).astype(np.int16)

        vc = np.zeros((NPAD, 6), dtype=np.float32)
        vc[:, 0:3] = vpad[core * BPC]
        vc[:, 3:6] = vpad[core * BPC + 1]

        msk = np.zeros((J2, J2), dtype=np.float32)
        msk[0:J, J:J2] = 1.0
        msk[J:J2, 0:J] = 1.0
        in_maps.append({
            "widx": widx,
            "jrt": np.ascontiguousarray(
                jrt.reshape(128, KCH * J).astype(bf16)),
            "vc": np.ascontiguousarray(
                vc.reshape(128, KCH * 6).astype(bf16)),
            "fsm": np.ascontiguousarray(
                fsm_all[bs].reshape(BPC * FPAD, E)),
            "msk": msk,
        })
    return in_maps


def kernel(collision_idxs, vertices, faces, joint_regressor):
    from concourse.bass_utils import run_bass_kernel_spmd

    nc = get_program()
    in_maps = make_in_maps(collision_idxs, vertices, faces, joint_regressor)
    res = run_bass_kernel_spmd(nc, in_maps, core_ids=list(range(NCORES)))
    num = 0.0
    den = 0.0
    for r in res.results:
        o = np.asarray(r["out"], dtype=np.float64).reshape(J2, 4)
        num += o[:, 0].sum() + o[:, 2].sum()
        den += o[J:J2, 1].sum() + o[J:J2, 3].sum()
    if den > 0:
        val = num / max(den, 1.0)
    else:
        val = 0.0
    return np.float32(val)


# revision 36
# speedup vs baseline: 1.0082x; 1.0082x over previous
"""Trainium2 Bass kernel for nn_CollisonToJointLoss.

Math restructure (same identity as the earlier version): jr >= 0, so where
both gathered scores are nonzero, |intr_s + recv_s| = intr_s + recv_s, and

    num_b = <D_b, Sum_c Sum_t [S|M]_intr^T [S|M]_recv  (TR + BL blocks)>
    den_b = sum(BR block),    with S_v = jr[v], M_v = (S_v > 0).

Key layout change vs the 104us version: the gather table is indexed BY FACE,
not by vertex.  Each 512B table row holds the face's three [S|M] vertex rows
([3 x 48] bf16 = 288B used).  One collision side therefore costs ONE gather
descriptor instead of 1 (face->verts) + 3 (vert->jr) descriptors of 256B
each: 8192 descriptors total instead of 32768, i.e. ~11.7us of DMA instead
of ~46.6us under the 22.76ns/descriptor DMA cost (256B and 512B descriptors
cost the same; the floor is at work per descriptor, not bytes).

The gather index array is host-prepared in the HW wrapped layout
([16, n/16] int16, replicated across the eight 16-partition groups), with
invalid collisions (cf[:,0] < 0) redirected to an all-zero table row and the
per-batch table base (+b*FPAD) folded in.  The gather runs as 4 chunks of
2048 descriptors so PE accumulation overlaps later chunks, and batch 0's
final reduction overlaps batch 1's gather.

Sharding: data-parallel over batch B: 8 cores x 2 batches.  Each core
returns partial (num, den); host sums and finishes the mean.
"""

import numpy as np

B, C, N, F, J = 16, 2048, 6890, 13776, 24
NCORES = 8
BPC = B // NCORES          # batches per core
NPAD = 6912                # 128 * 54  (jr/verts padded with zero rows)
KCH = NPAD // 128          # 54 chunks for the joints matmul
FPAD = F + 1               # table rows per batch incl. zero-face row
J2 = 2 * J                 # 48
E = 256                    # table row width in bf16 elems (512B stride)
NIDX = BPC * C * 2         # 8192 gather descriptors per core
NCHUNK = 4
CIDX = NIDX // NCHUNK      # 2048 descriptors per gather chunk

_CACHE = {}


def _build_program():
    import concourse.bass as bass
    import concourse.tile as tile
    from concourse import bacc, mybir
    from concourse.masks import make_identity

    f32 = mybir.dt.float32
    bf16 = mybir.dt.bfloat16
    i16 = mybir.dt.int16
    Alu = mybir.AluOpType

    nc = bacc.Bacc("TRN2", target_bir_lowering=False, debug=False,
                   num_swdge_queues=2)

    widx_d = nc.dram_tensor("widx", [128, NIDX // 16], i16,
                            kind="ExternalInput").ap()
    jrt_d = nc.dram_tensor("jrt", [128, KCH * J], bf16,
                           kind="ExternalInput").ap()
    vc_d = nc.dram_tensor("vc", [128, KCH * 6], bf16,
                          kind="ExternalInput").ap()
    fsm_d = nc.dram_tensor("fsm", [BPC * FPAD, E], bf16,
                           kind="ExternalInput").ap()
    msk_d = nc.dram_tensor("msk", [J2, J2], f32, kind="ExternalInput").ap()
    out_d = nc.dram_tensor("out", [J2, 4], f32, kind="ExternalOutput").ap()

    with tile.TileContext(nc) as tc:
        with tc.tile_pool(name="sb", bufs=1) as sb, \
             tc.tile_pool(name="pp", bufs=1, space="PSUM") as pp:

            # ---- bulk loads (widx first: it gates the gather pipeline).
            # The first chunk's index columns load separately so its
            # descriptor generation starts ~270ns earlier.
            WIDX = sb.tile([128, NIDX // 16], i16)
            nc.sync.dma_start(out=WIDX[:, 0:80], in_=widx_d[:, 0:80])
            nc.sync.dma_start(out=WIDX[:, 80:], in_=widx_d[:, 80:])
            JT = sb.tile([128, KCH, J], bf16)
            nc.sync.dma_start(out=JT[:].rearrange("p k j -> p (k j)"),
                              in_=jrt_d)
            VC = sb.tile([128, KCH, 6], bf16)
            nc.sync.dma_start(out=VC[:].rearrange("p k d -> p (k d)"),
                              in_=vc_d)

            MSK = sb.tile([J2, J2], f32)
            nc.sync.dma_start(out=MSK[:], in_=msk_d)
            ident = sb.tile([128, 128], f32)
            make_identity(nc, ident[:])

            # ---- joints = jr^T-chunks contracted with verts ----------------
            J6p = pp.tile([J, 6], f32)
            for k in range(KCH):
                nc.tensor.matmul(out=J6p[:], lhsT=JT[:, k, :], rhs=VC[:, k, :],
                                 start=(k == 0), stop=(k == KCH - 1))
            j6 = sb.tile([J, 6], f32)
            nc.vector.tensor_copy(out=j6[:], in_=J6p[:])

            # jt_b^T [3, 24] duplicated to [3, 48] so the squared-distance
            # matrix lands on all 48 partitions directly (D48 blocks = D).
            jtp = pp.tile([3, J2], f32)
            for b in range(BPC):
                nc.tensor.transpose(out=jtp[:, J * b:J * b + J],
                                    in_=j6[:, 3 * b:3 * b + 3],
                                    identity=ident[:J, :J])
            jtd = [sb.tile([3, J2], f32, name=f"jtd{b}") for b in range(BPC)]
            sqd = [sb.tile([3, J2], f32, name=f"sqd{b}") for b in range(BPC)]
            jtm2 = [sb.tile([3, J2], f32, name=f"jtm2{b}") for b in range(BPC)]
            ones3_48 = sb.tile([3, J2], f32)
            nc.vector.memset(ones3_48[:], 1.0)
            for b in range(BPC):
                nc.vector.tensor_copy(out=jtd[b][:, 0:J],
                                      in_=jtp[:, J * b:J * b + J])
                nc.vector.tensor_copy(out=jtd[b][:, J:J2],
                                      in_=jtp[:, J * b:J * b + J])
                nc.vector.tensor_mul(out=sqd[b][:], in0=jtd[b][:],
                                     in1=jtd[b][:])
                nc.vector.tensor_scalar_mul(out=jtm2[b][:], in0=jtd[b][:],
                                            scalar1=-2.0)

            # ---- DD_b: pairwise joint distances on 48 partitions, with the
            # diagonal blocks zeroed (only TR/BL of ACC contribute to num)
            G48 = pp.tile([J2, J2], f32)
            DD = [sb.tile([J2, J2], f32, name=f"DD{b}") for b in range(BPC)]
            for b in range(BPC):
                nc.tensor.matmul(out=G48[:], lhsT=jtm2[b][:], rhs=jtd[b][:],
                                 start=True, stop=False)
                nc.tensor.matmul(out=G48[:], lhsT=ones3_48[:], rhs=sqd[b][:],
                                 start=False, stop=False)
                nc.tensor.matmul(out=G48[:], lhsT=sqd[b][:], rhs=ones3_48[:],
                                 start=False, stop=True)
                nc.vector.tensor_scalar_max(out=DD[b][:], in0=G48[:],
                                            scalar1=0.0)
                nc.scalar.activation(out=DD[b][:], in_=DD[b][:],
                                     func=mybir.ActivationFunctionType.Sqrt)
                nc.vector.tensor_mul(out=DD[b][:], in0=DD[b][:], in1=MSK[:])

            # ---- chunked face-row gathers ---------------------------------
            # descriptor k = T*256 + side*128 + p; table row holds the face's
            # 3 [S|M] vertex rows at 48-elem offsets.  U slot = 2T + side.
            # Chunk sizes taper so the post-gather tail is short; desc-gen
            # (994 + 0.34/desc) stays ahead of the 1.42ns/desc transfers.
            U = sb.tile([128, NIDX // 128, E], bf16)
            bounds = [0, 1280, 3328, 5376, 7680, 8192]
            for ch in range(len(bounds) - 1):
                k0, k1 = bounds[ch], bounds[ch + 1]
                nc.gpsimd.dma_gather(
                    out_ap=U[:, k0 // 128:k1 // 128, :], in_ap=fsm_d,
                    idxs_ap=WIDX[:, k0 // 16:k1 // 16],
                    num_idxs=k1 - k0, num_idxs_reg=k1 - k0, elem_size=E,
                    single_packet=False, queue_num=ch % 2)

            # ---- accumulate ACC_b = Sum [S|M]_intr^T [S|M]_recv -----------
            # tiles T 0-15 are batch 0, 16-31 batch 1 (chunks in T order).
            # Each batch's reduction is emitted right after its last tile so
            # batch 0's reduction overlaps the remaining gather chunks.
            ACC = [pp.tile([J2, J2], f32, name=f"ACC{b}") for b in range(BPC)]
            VV = sb.tile([J2, 4], f32)
            nc.vector.memset(VV[:], 0.0)

            def reduction(b):
                NU = sb.tile([J2, J2], f32, name=f"NU{b}")
                nc.vector.tensor_mul(out=NU[:], in0=ACC[b][:], in1=DD[b][:])
                nc.vector.reduce_sum(out=VV[:, 2 * b:2 * b + 1], in_=NU[:],
                                     axis=mybir.AxisListType.X)
                nc.vector.reduce_sum(out=VV[:, 2 * b + 1:2 * b + 2],
                                     in_=ACC[b][:, J:J2],
                                     axis=mybir.AxisListType.X)

            started = [False, False]
            for T in range(32):
                b = T // 16
                for tau in range(3):
                    nc.tensor.matmul(
                        out=ACC[b][:],
                        lhsT=U[:, 2 * T + 1, J2 * tau:J2 * (tau + 1)],
                        rhs=U[:, 2 * T, J2 * tau:J2 * (tau + 1)],
                        start=not started[b],
                        stop=(T % 16 == 15 and tau == 2))
                    started[b] = True
                if T % 16 == 15:
                    reduction(b)
            nc.sync.dma_start(out=out_d, in_=VV[:])

    nc.compile()
    return nc


def get_program():
    if "nc" not in _CACHE:
        _CACHE["nc"] = _build_program()
    return _CACHE["nc"]


def make_in_maps(collision_idxs, vertices, faces, joint_regressor):
    """Host-side shard/layout prep. Returns list of per-core input dicts."""
    import ml_dtypes
    bf16 = ml_dtypes.bfloat16

    collision_idxs = np.asarray(collision_idxs)
    vertices = np.asarray(vertices)
    faces = np.asarray(faces).astype(np.int64)
    joint_regressor = np.asarray(joint_regressor)

    # jr^T padded, f32 for the joints matmul; [S|M] rows in bf16 for the table
    jrt = np.zeros((NPAD, J), dtype=np.float32)
    jrt[:N, :] = joint_regressor.T.astype(np.float32)
    sm = np.zeros((NPAD, J2), dtype=bf16)
    sm[:N, 0:J] = jrt[:N].astype(bf16)
    sm[:N, J:J2] = (jrt[:N] != 0).astype(bf16)

    # per-(batch, face) table row: 3 x [S|M] = 144 bf16, padded to 256
    fsm_all = np.zeros((B, FPAD, E), dtype=bf16)
    fsm_all[:, :F, 0:3 * J2] = sm[faces.reshape(B, F * 3)].reshape(B, F, 3 * J2)

    vpad = np.zeros((B, NPAD, 3), dtype=np.float32)
    vpad[:, :N, :] = vertices.astype(np.float32)

    # gather index values: valid ? clip(cf) : F (zero row), + b*FPAD
    cidx = collision_idxs.astype(np.int32)
    valid = cidx[:, :, 0] >= 0
    sel = np.empty((2, B, C), dtype=np.int32)      # side 0 = recv, 1 = intr
    sel[0] = np.where(valid, np.maximum(cidx[:, :, 0], 0), F)
    sel[1] = np.where(valid, np.maximum(cidx[:, :, 1], 0), F)

    # c(q, t, a) = q*128 + t*8 + a; descriptor k = T*256 + side*128 + 16a + q
    cgrid = (np.arange(16)[:, None, None] * 128 +
             np.arange(16)[None, :, None] * 8 +
             np.arange(8)[None, None, :])          # [q, t, a]

    in_maps = []
    for core in range(NCORES):
        bs = slice(core * BPC, (core + 1) * BPC)
        v = np.empty((2 * 16, 2, 8, 16), dtype=np.int32)   # [T, side, a, q]
        for bb in range(BPC):
            for side in range(2):
                g = sel[side, core * BPC + bb][cgrid]      # [q, t, a]
                v[bb * 16:(bb + 1) * 16, side] = (
                    bb * FPAD + g.transpose(1, 2, 0))      # [t, a, q]
        wrapped = v.reshape(NIDX // 16, 16).T              # [q, slot]
        widx = np.tile(wrapped, (8, 1)).astype(np.int16)

        vc = np.zeros((NPAD, 6), dtype=np.float32)
        vc[:, 0:3] = vpad[core * BPC]
        vc[:, 3:6] = vpad[core * BPC + 1]

        msk = np.zeros((J2, J2), dtype=np.float32)
        msk[0:J, J:J2] = 1.0
        msk[J:J2, 0:J] = 1.0
        in_maps.append({
            "widx": widx,
            "jrt": np.ascontiguousarray(
                jrt.reshape(128, KCH * J).astype(bf16)),
            "vc": np.ascontiguousarray(
                vc.reshape(128, KCH * 6).astype(bf16)),
            "fsm": np.ascontiguousarray(
                fsm_all[bs].reshape(BPC * FPAD, E)),
            "msk": msk,
        })
    return in_maps


def kernel(collision_idxs, vertices, faces, joint_regressor):
    from concourse.bass_utils import run_bass_kernel_spmd

    nc = get_program()
    in_maps = make_in_maps(collision_idxs, vertices, faces, joint_regressor)
    res = run_bass_kernel_spmd(nc, in_maps, core_ids=list(range(NCORES)))
    num = 0.0
    den = 0.0
    for r in res.results:
        o = np.asarray(r["out"], dtype=np.float64).reshape(J2, 4)
        num += o[:, 0].sum() + o[:, 2].sum()
        den += o[J:J2, 1].sum() + o[J:J2, 3].sum()
    if den > 0:
        val = num / max(den, 1.0)
    else:
        val = 0.0
    return np.float32(val)


# revision 40
# speedup vs baseline: 1.0110x; 1.0028x over previous
"""Trainium2 Bass kernel for nn_CollisonToJointLoss.

Math restructure (same identity as the earlier version): jr >= 0, so where
both gathered scores are nonzero, |intr_s + recv_s| = intr_s + recv_s, and

    num_b = <D_b, Sum_c Sum_t [S|M]_intr^T [S|M]_recv  (TR + BL blocks)>
    den_b = sum(BR block),    with S_v = jr[v], M_v = (S_v > 0).

Key layout change vs the 104us version: the gather table is indexed BY FACE,
not by vertex.  Each 512B table row holds the face's three [S|M] vertex rows
([3 x 48] bf16 = 288B used).  One collision side therefore costs ONE gather
descriptor instead of 1 (face->verts) + 3 (vert->jr) descriptors of 256B
each: 8192 descriptors total instead of 32768, i.e. ~11.7us of DMA instead
of ~46.6us under the 22.76ns/descriptor DMA cost (256B and 512B descriptors
cost the same; the floor is at work per descriptor, not bytes).

The gather index array is host-prepared in the HW wrapped layout
([16, n/16] int16, replicated across the eight 16-partition groups), with
invalid collisions (cf[:,0] < 0) redirected to an all-zero table row and the
per-batch table base (+b*FPAD) folded in.  The gather runs as 4 chunks of
2048 descriptors so PE accumulation overlaps later chunks, and batch 0's
final reduction overlaps batch 1's gather.

Sharding: data-parallel over batch B: 8 cores x 2 batches.  Each core
returns partial (num, den); host sums and finishes the mean.
"""

import numpy as np

B, C, N, F, J = 16, 2048, 6890, 13776, 24
NCORES = 8
BPC = B // NCORES          # batches per core
NPAD = 6912                # 128 * 54  (jr/verts padded with zero rows)
KCH = NPAD // 128          # 54 chunks for the joints matmul
FPAD = F + 1               # table rows per batch incl. zero-face row
J2 = 2 * J                 # 48
E = 256                    # table row width in bf16 elems (512B stride)
NIDX = BPC * C * 2         # 8192 gather descriptors per core
NCHUNK = 4
CIDX = NIDX // NCHUNK      # 2048 descriptors per gather chunk

_CACHE = {}

# gather chunk boundaries (descriptor index); first chunk sized so its
# desc-gen (994 fixed + 0.34/desc) finishes just as the DMA engines clear,
# last chunk small so the post-gather tail is short
BOUNDS = [0, 1280, 3328, 5376, 6912, 7936, 8192]


def _build_program():
    import concourse.bass as bass
    import concourse.tile as tile
    from concourse import bacc, mybir
    from concourse.masks import make_identity

    f32 = mybir.dt.float32
    bf16 = mybir.dt.bfloat16
    i16 = mybir.dt.int16
    Alu = mybir.AluOpType

    nc = bacc.Bacc("TRN2", target_bir_lowering=False, debug=False,
                   num_swdge_queues=2)

    widx_d = nc.dram_tensor("widx", [128, NIDX // 16], i16,
                            kind="ExternalInput").ap()
    jrt_d = nc.dram_tensor("jrt", [128, KCH * J], bf16,
                           kind="ExternalInput").ap()
    vc_d = nc.dram_tensor("vc", [128, KCH * 6], bf16,
                          kind="ExternalInput").ap()
    fsm_d = nc.dram_tensor("fsm", [BPC * FPAD, E], bf16,
                           kind="ExternalInput").ap()
    msk_d = nc.dram_tensor("msk", [J2, J2], f32, kind="ExternalInput").ap()
    out_d = nc.dram_tensor("out", [J2, 4], f32, kind="ExternalOutput").ap()

    with tile.TileContext(nc) as tc:
        with tc.tile_pool(name="sb", bufs=1) as sb, \
             tc.tile_pool(name="pp", bufs=1, space="PSUM") as pp:

            # ---- bulk loads (widx first: it gates the gather pipeline).
            # The first chunk's index columns load separately so its
            # descriptor generation starts ~270ns earlier.
            WIDX = sb.tile([128, NIDX // 16], i16)
            w1 = BOUNDS[1] // 16
            nc.sync.dma_start(out=WIDX[:, 0:w1], in_=widx_d[:, 0:w1])
            nc.sync.dma_start(out=WIDX[:, w1:], in_=widx_d[:, w1:])
            JT = sb.tile([128, KCH, J], bf16)
            nc.sync.dma_start(out=JT[:].rearrange("p k j -> p (k j)"),
                              in_=jrt_d)
            VC = sb.tile([128, KCH, 6], bf16)
            nc.sync.dma_start(out=VC[:].rearrange("p k d -> p (k d)"),
                              in_=vc_d)

            MSK = sb.tile([J2, J2], f32)
            nc.sync.dma_start(out=MSK[:], in_=msk_d)
            ident = sb.tile([128, 128], f32)
            make_identity(nc, ident[:])

            # ---- joints = jr^T-chunks contracted with verts ----------------
            J6p = pp.tile([J, 6], f32)
            for k in range(KCH):
                nc.tensor.matmul(out=J6p[:], lhsT=JT[:, k, :], rhs=VC[:, k, :],
                                 start=(k == 0), stop=(k == KCH - 1))
            j6 = sb.tile([J, 6], f32)
            nc.vector.tensor_copy(out=j6[:], in_=J6p[:])

            # jt_b^T [3, 24] duplicated to [3, 48] so the squared-distance
            # matrix lands on all 48 partitions directly (D48 blocks = D).
            jtp = pp.tile([3, J2], f32)
            for b in range(BPC):
                nc.tensor.transpose(out=jtp[:, J * b:J * b + J],
                                    in_=j6[:, 3 * b:3 * b + 3],
                                    identity=ident[:J, :J])
            jtd = [sb.tile([3, J2], f32, name=f"jtd{b}") for b in range(BPC)]
            sqd = [sb.tile([3, J2], f32, name=f"sqd{b}") for b in range(BPC)]
            jtm2 = [sb.tile([3, J2], f32, name=f"jtm2{b}") for b in range(BPC)]
            ones3_48 = sb.tile([3, J2], f32)
            nc.vector.memset(ones3_48[:], 1.0)
            for b in range(BPC):
                nc.vector.tensor_copy(out=jtd[b][:, 0:J],
                                      in_=jtp[:, J * b:J * b + J])
                nc.vector.tensor_copy(out=jtd[b][:, J:J2],
                                      in_=jtp[:, J * b:J * b + J])
                nc.vector.tensor_mul(out=sqd[b][:], in0=jtd[b][:],
                                     in1=jtd[b][:])
                nc.vector.tensor_scalar_mul(out=jtm2[b][:], in0=jtd[b][:],
                                            scalar1=-2.0)

            # ---- DD_b: pairwise joint distances on 48 partitions, with the
            # diagonal blocks zeroed (only TR/BL of ACC contribute to num)
            G48 = pp.tile([J2, J2], f32)
            DD = [sb.tile([J2, J2], f32, name=f"DD{b}") for b in range(BPC)]
            for b in range(BPC):
                nc.tensor.matmul(out=G48[:], lhsT=jtm2[b][:], rhs=jtd[b][:],
                                 start=True, stop=False)
                nc.tensor.matmul(out=G48[:], lhsT=ones3_48[:], rhs=sqd[b][:],
                                 start=False, stop=False)
                nc.tensor.matmul(out=G48[:], lhsT=sqd[b][:], rhs=ones3_48[:],
                                 start=False, stop=True)
                nc.vector.tensor_scalar_max(out=DD[b][:], in0=G48[:],
                                            scalar1=0.0)
                nc.scalar.activation(out=DD[b][:], in_=DD[b][:],
                                     func=mybir.ActivationFunctionType.Sqrt)
                nc.vector.tensor_mul(out=DD[b][:], in0=DD[b][:], in1=MSK[:])

            # ---- chunked face-row gathers ---------------------------------
            # descriptor k = T*256 + side*128 + p; table row holds the face's
            # 3 [S|M] vertex rows at 48-elem offsets.  U slot = 2T + side.
            # Chunk sizes taper so the post-gather tail is short; desc-gen
            # (994 + 0.34/desc) stays ahead of the 1.42ns/desc transfers.
            U = sb.tile([128, NIDX // 128, E], bf16)
            bounds = BOUNDS
            for ch in range(len(bounds) - 1):
                k0, k1 = bounds[ch], bounds[ch + 1]
                nc.gpsimd.dma_gather(
                    out_ap=U[:, k0 // 128:k1 // 128, :], in_ap=fsm_d,
                    idxs_ap=WIDX[:, k0 // 16:k1 // 16],
                    num_idxs=k1 - k0, num_idxs_reg=k1 - k0, elem_size=E,
                    single_packet=False, queue_num=ch % 2)

            # ---- accumulate ACC_b = Sum [S|M]_intr^T [S|M]_recv -----------
            # tiles T 0-15 are batch 0, 16-31 batch 1 (chunks in T order).
            # Each batch's reduction is emitted right after its last tile so
            # batch 0's reduction overlaps the remaining gather chunks.
            ACC = [pp.tile([J2, J2], f32, name=f"ACC{b}") for b in range(BPC)]
            VV = sb.tile([J2, 4], f32)
            nc.vector.memset(VV[:], 0.0)

            def reduction(b):
                NU = sb.tile([J2, J2], f32, name=f"NU{b}")
                nc.vector.tensor_mul(out=NU[:], in0=ACC[b][:], in1=DD[b][:])
                nc.vector.reduce_sum(out=VV[:, 2 * b:2 * b + 1], in_=NU[:],
                                     axis=mybir.AxisListType.X)
                nc.vector.reduce_sum(out=VV[:, 2 * b + 1:2 * b + 2],
                                     in_=ACC[b][:, J:J2],
                                     axis=mybir.AxisListType.X)

            started = [False, False]
            for T in range(32):
                b = T // 16
                for tau in range(3):
                    nc.tensor.matmul(
                        out=ACC[b][:],
                        lhsT=U[:, 2 * T + 1, J2 * tau:J2 * (tau + 1)],
                        rhs=U[:, 2 * T, J2 * tau:J2 * (tau + 1)],
                        start=not started[b],
                        stop=(T % 16 == 15 and tau == 2))
                    started[b] = True
                if T % 16 == 15:
                    reduction(b)
            nc.sync.dma_start(out=out_d, in_=VV[:])

    nc.compile()
    return nc


def get_program():
    if "nc" not in _CACHE:
        _CACHE["nc"] = _build_program()
    return _CACHE["nc"]


def make_in_maps(collision_idxs, vertices, faces, joint_regressor):
    """Host-side shard/layout prep. Returns list of per-core input dicts."""
    import ml_dtypes
    bf16 = ml_dtypes.bfloat16

    collision_idxs = np.asarray(collision_idxs)
    vertices = np.asarray(vertices)
    faces = np.asarray(faces).astype(np.int64)
    joint_regressor = np.asarray(joint_regressor)

    # jr^T padded, f32 for the joints matmul; [S|M] rows in bf16 for the table
    jrt = np.zeros((NPAD, J), dtype=np.float32)
    jrt[:N, :] = joint_regressor.T.astype(np.float32)
    sm = np.zeros((NPAD, J2), dtype=bf16)
    sm[:N, 0:J] = jrt[:N].astype(bf16)
    sm[:N, J:J2] = (jrt[:N] != 0).astype(bf16)

    # per-(batch, face) table row: 3 x [S|M] = 144 bf16, padded to 256
    fsm_all = np.zeros((B, FPAD, E), dtype=bf16)
    fsm_all[:, :F, 0:3 * J2] = sm[faces.reshape(B, F * 3)].reshape(B, F, 3 * J2)

    vpad = np.zeros((B, NPAD, 3), dtype=np.float32)
    vpad[:, :N, :] = vertices.astype(np.float32)

    # gather index values: valid ? clip(cf) : F (zero row), + b*FPAD
    cidx = collision_idxs.astype(np.int32)
    valid = cidx[:, :, 0] >= 0
    sel = np.empty((2, B, C), dtype=np.int32)      # side 0 = recv, 1 = intr
    sel[0] = np.where(valid, np.maximum(cidx[:, :, 0], 0), F)
    sel[1] = np.where(valid, np.maximum(cidx[:, :, 1], 0), F)

    # c(q, t, a) = q*128 + t*8 + a; descriptor k = T*256 + side*128 + 16a + q
    cgrid = (np.arange(16)[:, None, None] * 128 +
             np.arange(16)[None, :, None] * 8 +
             np.arange(8)[None, None, :])          # [q, t, a]

    in_maps = []
    for core in range(NCORES):
        bs = slice(core * BPC, (core + 1) * BPC)
        v = np.empty((2 * 16, 2, 8, 16), dtype=np.int32)   # [T, side, a, q]
        for bb in range(BPC):
            for side in range(2):
                g = sel[side, core * BPC + bb][cgrid]      # [q, t, a]
                v[bb * 16:(bb + 1) * 16, side] = (
                    bb * FPAD + g.transpose(1, 2, 0))      # [t, a, q]
        wrapped = v.reshape(NIDX // 16, 16).T              # [q, slot]
        widx = np.tile(wrapped, (8, 1)).astype(np.int16)

        vc = np.zeros((NPAD, 6), dtype=np.float32)
        vc[:, 0:3] = vpad[core * BPC]
        vc[:, 3:6] = vpad[core * BPC + 1]

        msk = np.zeros((J2, J2), dtype=np.float32)
        msk[0:J, J:J2] = 1.0
        msk[J:J2, 0:J] = 1.0
        in_maps.append({
            "widx": widx,
            "jrt": np.ascontiguousarray(
                jrt.reshape(128, KCH * J).astype(bf16)),
            "vc": np.ascontiguousarray(
                vc.reshape(128, KCH * 6).astype(bf16)),
            "fsm": np.ascontiguousarray(
                fsm_all[bs].reshape(BPC * FPAD, E)),
            "msk": msk,
        })
    return in_maps


def kernel(collision_idxs, vertices, faces, joint_regressor):
    from concourse.bass_utils import run_bass_kernel_spmd

    nc = get_program()
    in_maps = make_in_maps(collision_idxs, vertices, faces, joint_regressor)
    res = run_bass_kernel_spmd(nc, in_maps, core_ids=list(range(NCORES)))
    num = 0.0
    den = 0.0
    for r in res.results:
        o = np.asarray(r["out"], dtype=np.float64).reshape(J2, 4)
        num += o[:, 0].sum() + o[:, 2].sum()
        den += o[J:J2, 1].sum() + o[J:J2, 3].sum()
    if den > 0:
        val = num / max(den, 1.0)
    else:
        val = 0.0
    return np.float32(val)
